# revision 11
# baseline (speedup 1.0000x reference)
"""Trainium2 Bass kernel for nn_BiLSTM: 2-layer BiLSTM (B=64,T=512,D=64,H=128) + FC.

Sharding: data-parallel over batch across 8 NeuronCores (8 samples/core).

Design (split-dir, in-phase lockstep, minimal-chain LSTM cell, ~1867ns/step):
  Per layer, fwd and bwd run as two independent recurrent chains advanced in
  lockstep; per step s each dir does:
    PE : 4 gate matmuls  gates += Whh_g @ h~(s-1)    [128,8] into PSUM bank
    ACT: sg = Sigmoid(bank[32 cols])          -> u[par][0:32]   (i,f,g,o x 8)
    DVE: PQ = (in0 - .5) * in1  where in0=[g~|C], in1=[i~|f~]   -> [p|q]
    DVE: C' = (p + .5) + q                    -> u[par^1][32:40]
    ACT: v^ = Sigmoid(4C' - 2)                -> vhat
    DVE: h~ = (v^ - .5) * o~                  -> X[l] column (bf16)
  State: C = c/2 + 0.5 kept adjacent to the sigma outputs so PQ is one
  strided-AP op.  h~ = h/2; consumers (Whh, Wih_l1, Wfc) pre-scaled by 2.
  g-gate rows pre-scaled by 2 so one Sigmoid covers tanh(g) via 2sig(2g)-1.

  Engine queue order per step: PE [mmf x4, mmb x4, pregate piece]
                               ACT [sgf, sgb, scf, scb]
                               DVE [PQf, ms, Cf, PQb, ms, Cb, hf, hb]
  The memset spacers satisfy the DVE gap-0 RAW hazard AND keep Cf from
  queueing behind PQb's semaphore wait (fwd chain must not detour through
  the bwd gates).  Pregates (bias+x-part) accumulate into 2 PSUM banks/dir
  (16 steps each), staged 1 chunk ahead in small pieces during PE idle; no
  explicit waits needed (transitively ordered through the h-dependency).
  x is transposed to [64d, t*8+b] bf16 on the host; FC bias rides the
  PSUM->SBUF copy as an ACT Identity per-partition bias AP; outputs DMA out
  per 1024-col slice overlapping later FC chunks.
Host: reshape y -> [8,T,64] per core, concat cores -> [64,T,64].
"""
import sys
sys.path.insert(0, "/opt/trn_rl_repo")
import numpy as np
import ml_dtypes

import concourse.bass as bass
from concourse import mybir
from concourse.bass_utils import run_bass_kernel_spmd

F32 = mybir.dt.float32
BF16 = mybir.dt.bfloat16
BF = ml_dtypes.bfloat16
AluOp = mybir.AluOpType
ActFn = mybir.ActivationFunctionType

H = 128
PT = {"i": 0, "f": 1, "g": 2, "o": 3}   # PyTorch row-block order
GO = ["i", "f", "g", "o"]               # PSUM/u col-block order (8 cols each)
DIRS = ("f", "b")


def ap_of(t, off, dims):
    base = t[:] if not isinstance(t, bass.AP) else t
    return bass.AP(tensor=base.tensor, offset=base.offset + off, ap=list(dims))


def pstride(t):
    base = t[:] if not isinstance(t, bass.AP) else t
    return base.ap[0][0]


def build_nc(T=512):
    assert T % 16 == 0
    NTOK = T * 8
    NCH = T // 16                      # pregate chunks (16 steps each)
    nc = bass.Bass("TRN2", target_bir_lowering=False, debug=False)

    # register -2.0 const AP (sigma-cell bias), same pattern as Bass.__init__
    _c = nc.alloc_sbuf_tensor("const-f32-neg2", [128, 1], F32)
    nc.gpsimd.memset(_c.ap(), -2.0)
    nc.const_aps.aps[(F32, -2.0)] = _c.ap()
    nc.all_engine_barrier()

    # ---------------- DRAM I/O (batched packs to cut DMA descriptors) ----
    # x0: host-pretransposed input  [64 d, t*8+b]  bf16
    x0_d = nc.dram_tensor("x0", [64, NTOK], BF16, kind="ExternalInput")
    # wpack cols: whh0f whh0b whh1f whh1b wih1af wih1ab wih1bf wih1bb (x512)
    wpack_d = nc.dram_tensor("wpack", [128, 4096], BF16, kind="ExternalInput")
    wih0p_d = nc.dram_tensor("wih0p", [64, 1024], BF16, kind="ExternalInput")
    # bpack cols: b4(0f) b4(0b) b4(1f) b4(1b) (x128), mask4 (512)
    bpack_d = nc.dram_tensor("bpack", [4, 1024], BF16, kind="ExternalInput")
    # fpack cols: wfca(64) wfcb(64) zero8(8)
    fpack_d = nc.dram_tensor("fpack", [128, 136], BF16, kind="ExternalInput")
    bfc64_d = nc.dram_tensor("bfc64", [64, 1], F32, kind="ExternalInput")
    y_d = nc.dram_tensor("y", [64, NTOK], F32, kind="ExternalOutput")

    # ---------------- SBUF ----------------
    sb = nc.alloc_sbuf_tensor
    X0 = sb("X0", [64, NTOK], BF16)
    XL = {1: sb("XL1", [128, 2 * NTOK], BF16), 2: sb("XL2", [128, 2 * NTOK], BF16)}
    y_s = sb("y_s", [64, NTOK], F32)

    wpack = sb("wpack_s", [128, 4096], BF16)
    wih0p = sb("wih0p_s", [64, 1024], BF16)
    bpack = sb("bpack_s", [4, 1024], BF16)
    fpack = sb("fpack_s", [128, 136], BF16)
    bfc64 = sb("bfc64_s", [64, 1], F32)

    WOFF = {(0, "f"): 0, (0, "b"): 512, (1, "f"): 1024, (1, "b"): 1536}
    W1A = {"f": 2048, "b": 2560}
    W1B = {"f": 3072, "b": 3584}
    B4OFF = {(0, "f"): 0, (0, "b"): 128, (1, "f"): 256, (1, "b"): 384}

    def whh_slice(l, d, g):
        return ap_of(wpack, WOFF[(l, d)] + g * 128, [[pstride(wpack), 128], [1, 128]])

    def zero8_ap():
        return ap_of(fpack, 128, [[pstride(fpack), 128], [1, 8]])

    # u[(d,par)]: cols 0:32 = sigma(gates) [i f g o]; cols 32:40 = C state
    u = {(d, p): sb(f"u_{d}{p}", [128, 40], F32) for d in DIRS for p in (0, 1)}
    pq = {d: sb(f"pq_{d}", [128, 16], F32) for d in DIRS}
    vhat = {d: sb(f"vhat_{d}", [128, 8], F32) for d in DIRS}
    dummy = sb("dummy_sp", [128, 1], F32)

    gb = {(d, i): nc.alloc_psum_tensor(f"gb_{d}{i}", [128, 512], F32)
          for d in DIRS for i in (0, 1)}
    tbank = [nc.alloc_psum_tensor(f"tb{i}", [64, 512], F32) for i in range(4)]

    sem_in = nc.alloc_semaphore("sem_in")
    s_mm = nc.alloc_semaphore("s_mm")
    s_act = nc.alloc_semaphore("s_act")
    s_dve = nc.alloc_semaphore("s_dve")
    s_out = nc.alloc_semaphore("s_out")
    cnt = {"mm": 0, "act": 0, "dve": 0}
    sems = {"mm": s_mm, "act": s_act, "dve": s_dve}

    def W(eng, which, val):
        eng.wait_ge(sems[which], val)

    # pre-warm the sigmoid activation table set while input DMAs stream
    # (the first ACT instruction otherwise pays the ~2.7us table load on the
    # critical path; Copy lives in every set so it won't trigger a reload)
    nc.scalar.activation(dummy[:, :], dummy[:, :], ActFn.Sigmoid)

    def inc(ins, which):
        ins.then_inc(sems[which], 1)
        cnt[which] += 1
        return cnt[which]

    # ---------------- input DMAs ----------------
    n_dma = 0

    def dma(dst, src):
        nonlocal n_dma
        nc.sync.dma_start(out=dst, in_=src).then_inc(sem_in, 16)
        n_dma += 1

    # spread the DMAs across engines: descriptor generation costs ~0.6-0.9us
    # per dma_start on the issuing engine, so serializing all of them on the
    # sync engine delays ring start; parallel rings also overlap transfers
    dma(X0[:, :], x0_d[:, :])
    nc.gpsimd.dma_start(out=wpack[:, :], in_=wpack_d[:, :]).then_inc(sem_in, 16)
    nc.scalar.dma_start(out=wih0p[:, :], in_=wih0p_d[:, :]).then_inc(sem_in, 16)
    nc.scalar.dma_start(out=bpack[:, :], in_=bpack_d[:, :]).then_inc(sem_in, 16)
    nc.scalar.dma_start(out=fpack[:, :], in_=fpack_d[:, :]).then_inc(sem_in, 16)
    n_dma += 4
    dma(bfc64[:, :], bfc64_d[:, :])

    # ---------------- BiLSTM layers ----------------
    def pregate_bias(l, d, c):
        if c >= NCH:
            return
        bank = gb[(d, c % 2)]
        b4ap = ap_of(bpack, B4OFF[(l, d)], [[pstride(bpack), 4], [1, 128]])
        mask4ap = ap_of(bpack, 512, [[pstride(bpack), 4], [1, 512]])
        nc.tensor.matmul(bank[:, 0:512], b4ap, mask4ap,
                         start=True, stop=False, skip_group_check=True)

    def pregate_part(l, d, c, pi):
        """x-part matmuls (part pi) for chunk c (steps 16c..16c+15) of dir d."""
        if c >= NCH:
            return
        bank = gb[(d, c % 2)]
        if l == 0:
            parts = [(wih0p, 0 if d == "f" else 512, X0, 64, 0)]
        else:
            parts = [(wpack, W1A[d], XL[1], 128, 0),
                     (wpack, W1B[d], XL[1], 128, NTOK)]
        if pi >= len(parts):
            return
        (wt, woff, Xsrc, K, xoff) = parts[pi]
        if d == "f":
            rhs = ap_of(Xsrc, xoff + c * 128, [[pstride(Xsrc), K], [1, 128]])
        else:
            # step j of chunk c handles time T-1-16c-j  -> negative stride
            rhs = ap_of(Xsrc, xoff + (T - 1 - 16 * c) * 8,
                        [[pstride(Xsrc), K], [-8, 16], [1, 8]])
        for g in range(4):
            dst = ap_of(bank, 8 * g, [[pstride(bank), 128], [32, 16], [1, 8]])
            lhsT = ap_of(wt, woff + g * 128, [[pstride(wt), K], [1, 128]])
            nc.tensor.matmul(dst, lhsT, rhs,
                             start=False, stop=False, skip_group_check=True)

    def pregate(l, d, c):
        pregate_bias(l, d, c)
        pregate_part(l, d, c, 0)
        pregate_part(l, d, c, 1)

    def layer(l, Xout):
        # barrier: inputs (X0 or XL1) fully written; weights DMA'd
        if l == 0:
            nc.tensor.wait_ge(sem_in, 16 * n_dma)
        W(nc.tensor, "act", cnt["act"])
        W(nc.tensor, "dve", cnt["dve"])
        # C state init: C = c/2 + 0.5 = 0.5 in u[(d,0)][:,32:40]
        for d in DIRS:
            nc.vector.memset(u[(d, 0)][:, 32:40], 0.5)
        for d in DIRS:
            pregate(l, d, 0)

        mm_done = {}
        sg_done = {}
        c_done = {}
        sc_done = {}
        h_done = {}

        for s in range(T):
            par = s % 2
            base = 32 * (s % 16)
            # ---- PE: rec matmuls fwd then bwd ----
            for d in DIRS:
                bank = gb[(d, (s // 16) % 2)]
                if s == 0:
                    rhs = zero8_ap()
                else:
                    W(nc.tensor, "dve", h_done[(d, s - 1)])
                    if d == "f":
                        rhs = Xout[:, (s - 1) * 8: s * 8]
                    else:
                        rhs = ap_of(Xout, NTOK + (T - s) * 8,
                                    [[pstride(Xout), 128], [1, 8]])
                last = None
                for g in range(4):
                    last = nc.tensor.matmul(
                        bank[:, base + 8 * g: base + 8 * g + 8],
                        whh_slice(l, d, g), rhs,
                        start=False, stop=True, skip_group_check=True)
                mm_done[(d, s)] = inc(last, "mm")
            # ---- PE: pregate pieces for chunk c+1 (safe: the target bank's
            # last sigma read was step 16c-1, ordered before via h-dep) ----
            j = s % 16
            if j == 0:
                pregate_bias(l, "f", s // 16 + 1)
            elif j == 1:
                pregate_part(l, "f", s // 16 + 1, 0)
            elif j == 2:
                pregate_part(l, "f", s // 16 + 1, 1)
            elif j == 3:
                pregate_bias(l, "b", s // 16 + 1)
            elif j == 4:
                pregate_part(l, "b", s // 16 + 1, 0)
            elif j == 5:
                pregate_part(l, "b", s // 16 + 1, 1)
            # ---- ACT: sigma over gates (both dirs) ----
            for d in DIRS:
                bank = gb[(d, (s // 16) % 2)]
                W(nc.scalar, "mm", mm_done[(d, s)])
                ins = nc.scalar.activation(u[(d, par)][:, 0:32],
                                           bank[:, base:base + 32], ActFn.Sigmoid)
                sg_done[(d, s)] = inc(ins, "act")
            # ---- DVE: [PQf, spacer, Cf, PQb, spacer, Cb] ----
            # The spacer (a) satisfies the DVE gap-0 RAW hazard between PQ and
            # C and (b) keeps Cf ahead of PQb's semaphore wait in the queue so
            # the fwd chain does not detour through the bwd gates.
            for d in DIRS:
                W(nc.vector, "act", sg_done[(d, s)])
                in0 = ap_of(u[(d, par)], 16, [[pstride(u[(d, par)]), 128], [16, 2], [1, 8]])
                in1 = ap_of(u[(d, par)], 0, [[pstride(u[(d, par)]), 128], [8, 2], [1, 8]])
                out = ap_of(pq[d], 0, [[pstride(pq[d]), 128], [8, 2], [1, 8]])
                nc.vector.scalar_tensor_tensor(out=out, in0=in0, scalar=0.5,
                                               in1=in1, op0=AluOp.subtract,
                                               op1=AluOp.mult)
                nc.vector.memset(dummy[:, :], 0.0)
                ins = nc.vector.scalar_tensor_tensor(
                    out=u[(d, 1 - par)][:, 32:40], in0=pq[d][:, 0:8], scalar=0.5,
                    in1=pq[d][:, 8:16], op0=AluOp.add, op1=AluOp.add)
                c_done[(d, s)] = inc(ins, "dve")
            # ---- ACT: v^ = sigma(4C-2) ----
            for d in DIRS:
                W(nc.scalar, "dve", c_done[(d, s)])
                ins = nc.scalar.activation(vhat[d][:, :], u[(d, 1 - par)][:, 32:40],
                                           ActFn.Sigmoid, scale=4.0, bias=-2.0)
                sc_done[(d, s)] = inc(ins, "act")
            # ---- DVE: h~ = (v^-0.5)*o~ ----
            for d in DIRS:
                W(nc.vector, "act", sc_done[(d, s)])
                if d == "f":
                    dst = Xout[:, s * 8:(s + 1) * 8]
                else:
                    dst = ap_of(Xout, NTOK + (T - 1 - s) * 8,
                                [[pstride(Xout), 128], [1, 8]])
                ins = nc.vector.scalar_tensor_tensor(
                    out=dst, in0=vhat[d][:, :], scalar=0.5,
                    in1=u[(d, par)][:, 24:32], op0=AluOp.subtract, op1=AluOp.mult)
                h_done[(d, s)] = inc(ins, "dve")

    layer(0, XL[1])
    layer(1, XL[2])

    # ---------------- FC (+ per-pair output DMA overlap) ----------------
    # y = 2*Wfc @ [X2f; X2b] (+bias): two bf16 matmuls per 512-col chunk; the
    # per-row bias rides the PSUM->SBUF copy as an ACT Identity bias AP.
    W(nc.tensor, "act", cnt["act"])
    W(nc.tensor, "dve", cnt["dve"])
    wfca_ap = ap_of(fpack, 0, [[pstride(fpack), 128], [1, 64]])
    wfcb_ap = ap_of(fpack, 64, [[pstride(fpack), 128], [1, 64]])
    bias_ap = ap_of(bfc64, 0, [[pstride(bfc64), 64], [1, 1]])
    fc_copy = {}
    n_out = 0
    fc_starts = list(range(0, NTOK, 512))
    for i, st in enumerate(fc_starts):
        w = min(512, NTOK - st)
        bank = tbank[i % 4]
        if i >= 4:
            eng, c0 = fc_copy[i - 4]
            W(nc.tensor, eng, c0)
        nc.tensor.matmul(bank[0:64, 0:w], wfca_ap, XL[2][:, st:st + w],
                         start=True, stop=False, skip_group_check=True)
        ins = nc.tensor.matmul(bank[0:64, 0:w], wfcb_ap,
                               ap_of(XL[2], NTOK + st, [[pstride(XL[2]), 128], [1, w]]),
                               start=False, stop=True, skip_group_check=True)
        mmc = inc(ins, "mm")
        W(nc.scalar, "mm", mmc)
        ins = nc.scalar.activation(y_s[:, st:st + w], bank[0:64, 0:w],
                                   ActFn.Identity, bias=bias_ap)
        fc_copy[i] = ("act", inc(ins, "act"))
        if i % 2 == 1:
            # DMA out the finished 1024-col slice while later chunks compute
            nc.sync.wait_ge(s_act, fc_copy[i][1])
            nc.sync.dma_start(out=y_d[:, st - 512:st + w],
                              in_=y_s[:, st - 512:st + w]).then_inc(s_out, 16)
            n_out += 1
    if len(fc_starts) % 2 == 1:
        st = fc_starts[-1]
        w = min(512, NTOK - st)
        nc.sync.wait_ge(s_act, cnt["act"])
        nc.sync.dma_start(out=y_d[:, st:st + w],
                          in_=y_s[:, st:st + w]).then_inc(s_out, 16)
        n_out += 1

    nc.sync.wait_ge(s_out, 16 * n_out)
    return nc


# ====================== host-side prep & entry point ======================

def _to_bf(a):
    return np.asarray(a, dtype=np.float32).astype(BF)


def prep_weights(inputs):
    """Pre-scaled lhsT tensors per the v2 formulation."""
    out = {}
    for l in (0, 1):
        xin_scale = 1.0 if l == 0 else 2.0
        for dname, suf in (("f", ""), ("b", "r")):
            wih = np.asarray(inputs[f"w_ih_l{l}{suf}"], np.float32)   # [512, Din]
            whh = np.asarray(inputs[f"w_hh_l{l}{suf}"], np.float32)   # [512, 128]
            bsum = (np.asarray(inputs[f"b_ih_l{l}{suf}"], np.float32)
                    + np.asarray(inputs[f"b_hh_l{l}{suf}"], np.float32))
            blk_ih, blk_hh, b4 = [], [], np.zeros((4, 128), np.float32)
            for gi, G in enumerate(GO):
                rows = slice(PT[G] * 128, (PT[G] + 1) * 128)
                sG = 2.0 if G == "g" else 1.0
                blk_ih.append((sG * xin_scale * wih[rows]).T)   # [Din,128]
                blk_hh.append((sG * 2.0 * whh[rows]).T)         # [128,128]
                b4[gi] = sG * bsum[rows]
            wih_cat = np.concatenate(blk_ih, axis=1)            # [Din, 512]
            out[f"whh{l}{dname}"] = _to_bf(np.concatenate(blk_hh, axis=1))
            out[f"b4_{l}{dname}"] = _to_bf(b4)
            if l == 0:
                out[f"wih0{dname}"] = _to_bf(wih_cat)           # [64, 512]
            else:
                out[f"wih1a{dname}"] = _to_bf(wih_cat[0:128])
                out[f"wih1b{dname}"] = _to_bf(wih_cat[128:256])
    wfc = 2.0 * np.asarray(inputs["w_fc"], np.float32).T        # [256, 64]
    out["wfca"] = _to_bf(wfc[0:128])
    out["wfcb"] = _to_bf(wfc[128:256])
    out["bfc"] = np.asarray(inputs["b_fc"], np.float32).reshape(1, 64)
    return out


def _mask4_np():
    m = np.zeros((4, 512), np.float32)
    for g in range(4):
        for r in range(16):
            m[g, r * 32 + g * 8: r * 32 + g * 8 + 8] = 1.0
    return m.astype(BF)


_NC_CACHE = {}


def _get_nc(T):
    if T not in _NC_CACHE:
        _NC_CACHE[T] = build_nc(T)
    return _NC_CACHE[T]


def run_cores(inputs, T=512, n_cores=8, trace=False):
    x = np.asarray(inputs["x"], np.float32)
    per = 8
    wp = prep_weights(inputs)
    wpack = np.concatenate([wp["whh0f"], wp["whh0b"], wp["whh1f"], wp["whh1b"],
                            wp["wih1af"], wp["wih1ab"], wp["wih1bf"], wp["wih1bb"]],
                           axis=1)                                    # [128, 4096]
    wih0p = np.concatenate([wp["wih0f"], wp["wih0b"]], axis=1)        # [64, 1024]
    bpack = np.concatenate([wp["b4_0f"], wp["b4_0b"], wp["b4_1f"], wp["b4_1b"],
                            _mask4_np()], axis=1)                     # [4, 1024]
    fpack = np.concatenate([wp["wfca"], wp["wfcb"],
                            np.zeros((128, 8), BF)], axis=1)          # [128, 136]
    common = {
        "wpack": wpack, "wih0p": wih0p, "bpack": bpack, "fpack": fpack,
        "bfc64": wp["bfc"].reshape(64, 1),
    }
    in_maps = []
    for c in range(n_cores):
        m = dict(common)
        xc = x[c * per:(c + 1) * per, :T]                     # [8, T, 64]
        m["x0"] = np.ascontiguousarray(
            xc.transpose(2, 1, 0).reshape(64, T * 8)).astype(BF)
        in_maps.append(m)

    nc = _get_nc(T)
    res = run_bass_kernel_spmd(nc, in_maps, core_ids=list(range(n_cores)), trace=trace)
    outs = []
    for c in range(n_cores):
        yc = res.results[c]["y"]
        outs.append(yc.reshape(64, T, 8).transpose(2, 1, 0))
    return np.concatenate(outs, axis=0), res


def kernel(**inputs):
    y, _ = run_cores(inputs, T=512, n_cores=8)
    return y.astype(np.float32)
